# revision 31
# baseline (speedup 1.0000x reference)
"""DeepBoundaryTree retrieval-knn kernel for 8x Trainium2 (Bass/Tile).

Strategy: data-parallel over queries B across 8 cores. Each core:
  - runs the 2-100-100-30-2 MLP over its 1/8 shard of (padded) nodes on the
    tensor engine, AllGathers the 16B records [x', y', label, 0] and spreads
    them into a DRAM table with 4 records per 256B-strided row
    (int16-addressable by dma_gather),
  - gathers the 256 neighbor records per query with dma_gather, round-robined
    across 4 SWDGE queues (num_swdge_queues=4) so descriptor generation runs
    on all four Q7 core pairs concurrently (~4x gather throughput; desc-gen
    at ~8.4 ns/index is the kernel bottleneck). 1024 idx/call is the SWDGE
    ring limit (128 descs/engine/queue) - larger calls deadlock,
  - selects the right 16B record 1-of-4 via precomputed bit masks, computes
    pairwise distance + per-(b,d) softmax stats on vector/scalar engines,
  - computes the final-step class sums with 3 broadcast DVE ops on
    [128, C, K] (iota==label * exp weights, reduce over K),
  - writes its [1024, 100] output slice.

Host side only reshapes/shards inputs and precomputes integer index/layout
tensors derived from nbr_idx (gather rows, sub-slot select masks).
"""
import sys
import types
import numpy as np

B, N, D, K, C = 8192, 100000, 8, 32, 100
NCORES = 8
BC = B // NCORES            # queries per core
NTILES = BC // 128          # 128-query tiles per core
NPAD = 102400               # nodes padded to 8*128*100 (sapt%4==0 so any
                            # 4-slot boundary is whole-table-row aligned)
ROWS = NPAD // 4            # 25600 table rows (256B stride, 4 records each)
ROW_F32 = 64                # 64 f32 = 256B row stride
PER_CALL = 1024             # dma_gather indices per call (hard SWDGE ring
                            # limit: 128 descs/engine/queue, runtime-fixed)
CALLS_PER_TILE = (128 * D * K) // PER_CALL  # 32
APT = NPAD // 128           # 784 free-dim slots per partition in packed recs
EPS_PD = 1e-6
EPS_LOG = 1e-4


def _install_ntff_shim():
    try:
        import antenv
        if "antenv.axon_hooks" in sys.modules:
            return
        mod = types.ModuleType("antenv.axon_hooks")
        holder = [None]
        mod.set_axon_ntff_profile_hook = lambda h: holder.__setitem__(0, h)
        mod.get_axon_ntff_profile_hook = lambda: holder[0]
        sys.modules["antenv.axon_hooks"] = mod
        antenv.axon_hooks = mod
        import trn_agent_boot.trn_boot as tb
        mod.set_axon_ntff_profile_hook(
            tb._ntff_profile_via_ctypes("/opt/axon/libaxon_pjrt.so"))
    except Exception:
        pass


def dma_gather_raw(eng, mybir, out_ap, in_ap, idxs_ap, num_idxs, elem_size,
                   elem_step, queue_num=0):
    """bass.dma_gather without the 256B-element assert (DRAM src, no transpose)."""
    stride_bytes = elem_step * mybir.dt.size(in_ap.dtype)
    stride_bytes_256 = stride_bytes // 256
    assert stride_bytes_256 * 256 == stride_bytes and stride_bytes_256 < 256
    _in_ap = eng.lower_ap_dma(in_ap, for_custom_bir_dma=True)
    _idxs_ap = eng.lower_ap(idxs_ap)
    _out_ap = eng.lower_ap(out_ap)
    return eng.add_instruction(
        mybir.InstDMAGatherAnt(
            name=eng.bass.get_next_instruction_name(),
            ins=[*_in_ap, _idxs_ap, eng.lower_val_access(eng.to_reg(num_idxs))],
            outs=[_out_ap],
            transpose=False,
            num_idxs=num_idxs,
            elem_size=elem_size,
            stride_bytes_256=stride_bytes_256,
            gen_mode=0,
            single_packet=True,
            queue_num=queue_num,
            sbuf_tokens_per_rank=0,
            sbuf_free_dim_per_rank=0,
            sbuf_free_dim_pad_per_rank=0,
            sbuf_byte_offset=0,
        ))


def build_kernel(npad=NPAD, ntiles=NTILES):
    import concourse.bacc as bacc
    import concourse.mybir as mybir
    from concourse.tile import TileContext
    from concourse.masks import make_identity

    apt = npad // 128
    rows = npad // 4
    f32 = mybir.dt.float32
    AX = mybir.AxisListType
    AF = mybir.ActivationFunctionType
    OP = mybir.AluOpType

    nc = bacc.Bacc("TRN2", target_bir_lowering=False, debug=False,
                   num_swdge_queues=4)

    # ---- I/O ----
    shard = npad // 8
    sapt = shard // 128
    ndT = nc.dram_tensor("ndT", [2, shard], f32, kind="ExternalInput")
    lblfT = nc.dram_tensor("lblfT", [2, shard], f32, kind="ExternalInput")
    SA = 64                     # a-slots in the early AllGather: largest A
                                # whose collective still ends by MLP-end
                                # (measured: SA=72 ended 8us past MLP-end)
    SB = sapt - SA              # 32 a-slots in the late small AllGather
    packed_cA = nc.dram_tensor("packed_cA", [128, SA * 4], f32)
    packed_cB = nc.dram_tensor("packed_cB", [128, SB * 4], f32)
    packed_allA = nc.dram_tensor("packed_allA", [8, 128, SA * 4], f32,
                                 addr_space="Shared")
    packed_allB = nc.dram_tensor("packed_allB", [8, 128, SB * 4], f32,
                                 addr_space="Shared")
    xT = nc.dram_tensor("xT", [2, 128 * ntiles], f32, kind="ExternalInput")
    w1t = nc.dram_tensor("w1t", [2, 100], f32, kind="ExternalInput")
    w2t = nc.dram_tensor("w2t", [100, 100], f32, kind="ExternalInput")
    w3t = nc.dram_tensor("w3t", [100, 30], f32, kind="ExternalInput")
    w4t = nc.dram_tensor("w4t", [30, 2], f32, kind="ExternalInput")
    b1 = nc.dram_tensor("b1", [100, 1], f32, kind="ExternalInput")
    b2 = nc.dram_tensor("b2", [100, 1], f32, kind="ExternalInput")
    b3 = nc.dram_tensor("b3", [30, 1], f32, kind="ExternalInput")
    b4 = nc.dram_tensor("b4", [2, 1], f32, kind="ExternalInput")
    iota = nc.dram_tensor("iota", [128, C], f32, kind="ExternalInput")
    widx = nc.dram_tensor("widx", [ntiles, 16, 2048], mybir.dt.int16,
                          kind="ExternalInput")
    m0e = nc.dram_tensor("m0e", [ntiles, 128, 512], mybir.dt.int8, kind="ExternalInput")
    m1e = nc.dram_tensor("m1e", [ntiles, 128, 512], mybir.dt.int8, kind="ExternalInput")
    y = nc.dram_tensor("y", [128 * ntiles, C], f32, kind="ExternalOutput")

    table = nc.dram_tensor("table", [rows, ROW_F32], f32)

    with TileContext(nc) as tc:
        with tc.tile_pool(name="const", bufs=1) as cpool, \
             tc.tile_pool(name="mlp", bufs=4) as mpool, \
             tc.tile_pool(name="recs", bufs=1) as rpool, \
             tc.tile_pool(name="gath", bufs=2) as gpool, \
             tc.tile_pool(name="work", bufs=2) as wpool, \
             tc.tile_pool(name="psA", bufs=2, space="PSUM") as pA, \
             tc.tile_pool(name="psB", bufs=1, space="PSUM") as pB, \
             tc.tile_pool(name="pt", bufs=2, space="PSUM") as tpool:

            # ---- constants ----
            w1s = cpool.tile([2, 100], f32)
            w2s = cpool.tile([100, 100], f32)
            w3s = cpool.tile([100, 30], f32)
            w4s = cpool.tile([30, 2], f32)
            b1s = cpool.tile([100, 1], f32)
            b2s = cpool.tile([100, 1], f32)
            b3s = cpool.tile([30, 1], f32)
            b4s = cpool.tile([2, 1], f32)
            iot = cpool.tile([128, C], f32)
            id4 = cpool.tile([4, 4], f32)
            zb = cpool.tile([128, 1], f32)
            eb = cpool.tile([128, 1], f32)
            nc.vector.memset(zb[:], 0.0)
            nc.vector.memset(eb[:], float(EPS_LOG))
            for dst, src in ((w1s, w1t), (w2s, w2t), (w3s, w3t), (w4s, w4t),
                             (b1s, b1), (b2s, b2), (b3s, b3), (b4s, b4),
                             (iot, iota)):
                nc.sync.dma_start(out=dst[:], in_=src[:])
            make_identity(nc, id4[:])
            f32r = mybir.dt.float32r
            w1r = cpool.tile([2, 100], f32r)
            w2r = cpool.tile([100, 100], f32r)
            w3r = cpool.tile([100, 30], f32r)
            w4r = cpool.tile([30, 2], f32r)
            for dstr, srcr in ((w1r, w1s), (w2r, w2s), (w3r, w3s), (w4r, w4s)):
                nc.vector.tensor_copy(out=dstr[:], in_=srcr[:])

            xTs = cpool.tile([2, 128 * ntiles], f32)
            nc.sync.dma_start(out=xTs[:], in_=xT[:])

            recs = rpool.tile([128, sapt, 4], f32)
            qxT = rpool.tile([2, 128 * ntiles], f32)

            # ---- MLP over nodes (and queries appended) in 512-col steps ----
            def mlp_steps(src_dram, src_tile, ncols, out_cb, base=0):
                nsteps = (ncols + 511) // 512
                for s in range(nsteps):
                    w = min(512, ncols - 512 * s)
                    cs = slice(base + 512 * s, base + 512 * s + w)
                    if src_dram is not None:
                        nds = mpool.tile([2, 512], f32, tag="nds")
                        nc.sync.dma_start(out=nds[:, :w], in_=src_dram[:, cs])
                        srcv = nds[:, :w]
                    else:
                        srcv = src_tile[:, cs]
                    srcr = mpool.tile([2, 512], f32r, tag="srcr")
                    nc.vector.tensor_copy(out=srcr[:, :w], in_=srcv)
                    ps1 = pA.tile([100, 512], f32, tag="ps1")
                    nc.tensor.matmul(ps1[:, :w], w1r[:], srcr[:, :w],
                                     start=True, stop=True)
                    h1 = mpool.tile([100, 512], f32r, tag="h1")
                    nc.scalar.activation(h1[:, :w], ps1[:, :w], AF.Relu,
                                         bias=b1s[:, 0:1])
                    ps2 = pA.tile([100, 512], f32, tag="ps2")
                    nc.tensor.matmul(ps2[:, :w], w2r[:], h1[:, :w],
                                     start=True, stop=True)
                    h2 = mpool.tile([100, 512], f32r, tag="h2")
                    # relu on DVE: (ps2 + b2) max 0
                    nc.vector.tensor_scalar(
                        out=h2[:, :w], in0=ps2[:, :w], scalar1=b2s[:, 0:1],
                        scalar2=0.0, op0=OP.add, op1=OP.max)
                    ps3 = pB.tile([30, 512], f32, tag="ps3")
                    nc.tensor.matmul(ps3[:, :w], w3r[:], h2[:, :w],
                                     start=True, stop=True)
                    h3 = mpool.tile([30, 512], f32r, tag="h3")
                    nc.scalar.activation(h3[:, :w], ps3[:, :w], AF.Relu,
                                         bias=b3s[:, 0:1])
                    ps4 = pB.tile([2, 512], f32, tag="ps4")
                    nc.tensor.matmul(ps4[:, :w], w4r[:], h3[:, :w],
                                     start=True, stop=True)
                    out_cb(s, ps4, w)

            # nodes: build records, transposes lag one step so PE never
            # stalls on the DVE record build of the same step
            pending = []

            def flush_rec(rec512, s, w):
                pst = tpool.tile([128, 16], f32, tag="pst")
                nb = w // 128
                for j in range(nb):
                    nc.tensor.transpose(pst[:, 4 * j:4 * (j + 1)],
                                        rec512[:, 128 * j:128 * (j + 1)],
                                        id4[:])
                nc.vector.tensor_copy(
                    out=recs[:, 4 * s:4 * s + nb, :].rearrange(
                        "p a c -> p (a c)"),
                    in_=pst[:, :4 * nb])

            def node_out(s, ps4, w):
                rec512 = mpool.tile([4, 512], f32, tag="rec512")
                # x' = x + b4 + eps  (vector: (ps4 + b4) + eps)
                nc.vector.tensor_scalar(
                    out=rec512[0:2, :w], in0=ps4[:, :w], scalar1=b4s[:, 0:1],
                    scalar2=float(EPS_PD), op0=OP.add, op1=OP.add)
                nc.sync.dma_start(out=rec512[2:4, :w],
                                  in_=lblfT[:, 512 * s:512 * s + w])
                pending.append((rec512, s, w))
                if len(pending) >= 2:
                    flush_rec(*pending.pop(0))

            # part A (17 steps = 68 slots): AllGather it while the last 8
            # MLP steps still run, so its ~20us fixed cost is hidden
            stA = SA // 4
            mlp_steps(ndT, None, stA * 512, node_out)
            while pending:
                flush_rec(*pending.pop(0))
            nc.sync.dma_start(out=packed_cA[:],
                              in_=recs[:, 0:SA, :].rearrange(
                                  "p a c -> p (a c)"))
            nc.gpsimd.collective_compute(
                "AllGather", OP.bypass,
                replica_groups=[list(range(8))],
                ins=[packed_cA[:]],
                outs=[packed_allA[:]],
            )
            tb_v = table[:, 0:16].rearrange("(c p v) e -> c p v e", c=8,
                                            p=128)
            pkA = packed_allA[:].rearrange("c p (r e) -> c p r e", e=16)
            rA = SA // 8                # rows per half-chunk spread DMA
            for i in range(8):
                for h in range(2):
                    nc.gpsimd.dma_start(
                        out=tb_v[i, :, rA * h:rA * (h + 1), :],
                        in_=pkA[i, :, rA * h:rA * (h + 1), :])

            mlp_steps(ndT, None, shard - stA * 512,
                      lambda s, ps4, w: node_out(s + stA, ps4, w),
                      base=stA * 512)
            while pending:
                flush_rec(*pending.pop(0))
            nc.sync.dma_start(out=packed_cB[:],
                              in_=recs[:, SA:, :].rearrange(
                                  "p a c -> p (a c)"))
            nc.gpsimd.collective_compute(
                "AllGather", OP.bypass,
                replica_groups=[list(range(8))],
                ins=[packed_cB[:]],
                outs=[packed_allB[:]],
            )
            pkB = packed_allB[:].rearrange("c p (r e) -> c p r e", e=16)
            for i in range(8):
                nc.sync.dma_start(out=tb_v[i, :, SA // 4:sapt // 4, :],
                                  in_=pkB[i, :, :, :])

            # queries: qxT = out + b4 (no eps)
            def query_out(s, ps4, w):
                nc.vector.tensor_scalar(
                    out=qxT[:, 512 * s:512 * s + w], in0=ps4[:, :w],
                    scalar1=b4s[:, 0:1], scalar2=None, op0=OP.add)

            mlp_steps(None, xTs, 128 * ntiles, query_out)

            # per-tile query coords [128, 2]
            id2 = cpool.tile([2, 2], f32)
            make_identity(nc, id2[:])

            # ---- main loop over 128-query tiles ----
            for t in range(ntiles):
                qps = tpool.tile([128, 16], f32, tag="pst")
                nc.tensor.transpose(qps[:, 0:2], qxT[:, 128 * t:128 * (t + 1)],
                                    id2[:])
                qx = wpool.tile([128, 2], f32, tag="qx")
                nc.vector.tensor_copy(out=qx[:], in_=qps[:, 0:2])

                idx_t = gpool.tile([128, 2048], mybir.dt.int16, tag="idx")
                for g in range(8):
                    nc.sync.dma_start(out=idx_t[16 * g:16 * (g + 1), :],
                                      in_=widx[t, :, :])
                m0 = wpool.tile([128, 256, 2], mybir.dt.int8, tag="m0")
                m1 = wpool.tile([128, 256, 2], mybir.dt.int8, tag="m1")
                nc.sync.dma_start(out=m0[:].rearrange("p s c -> p (s c)"),
                                  in_=m0e[t, :, :])
                nc.sync.dma_start(out=m1[:].rearrange("p s c -> p (s c)"),
                                  in_=m1e[t, :, :])

                gt = gpool.tile([128, 256, 16], f32, tag="gt")
                GC = PER_CALL // 128   # gt cols per call
                IC = PER_CALL // 16    # idx cols per call
                s01t = wpool.tile([128, 256, 2], f32, tag="s01")
                s23t = wpool.tile([128, 256, 2], f32, tag="s23")
                rect = wpool.tile([128, 256, 2], f32, tag="rec")
                dx = wpool.tile([128, 256], f32, tag="dx")
                dy = wpool.tile([128, 256], f32, tag="dy")
                d2 = wpool.tile([128, 256], f32, tag="d2")
                dist = wpool.tile([128, 256], f32, tag="dist")
                ex = wpool.tile([128, 256], f32, tag="ex")
                HB = CALLS_PER_TILE // 2

                def half(h):
                    # gathers for this half, then its select/dist/exp so only
                    # the second half's chain is exposed after the last call
                    for q in range(HB * h, HB * (h + 1)):
                        dma_gather_raw(
                            nc.gpsimd, mybir,
                            gt[:, GC * q:GC * (q + 1), :],
                            table[:, 0:16],
                            idx_t[:, IC * q:IC * (q + 1)],
                            num_idxs=PER_CALL, elem_size=16,
                            elem_step=ROW_F32, queue_num=q % 4)
                    cs = slice(128 * h, 128 * (h + 1))
                    nc.vector.select(s01t[:, cs, :], m0[:, cs, :],
                                     gt[:, cs, 4:6], gt[:, cs, 0:2])
                    nc.vector.select(s23t[:, cs, :], m0[:, cs, :],
                                     gt[:, cs, 12:14], gt[:, cs, 8:10])
                    nc.vector.select(rect[:, cs, :], m1[:, cs, :],
                                     s23t[:, cs, :], s01t[:, cs, :])
                    nc.vector.tensor_scalar(out=dx[:, cs], in0=rect[:, cs, 0],
                                            scalar1=qx[:, 0:1], scalar2=None,
                                            op0=OP.subtract)
                    nc.vector.tensor_scalar(out=dy[:, cs], in0=rect[:, cs, 1],
                                            scalar1=qx[:, 1:2], scalar2=None,
                                            op0=OP.subtract)
                    nc.vector.tensor_tensor(out=d2[:, cs], in0=dx[:, cs],
                                            in1=dx[:, cs], op=OP.mult)
                    nc.vector.tensor_tensor(out=dy[:, cs], in0=dy[:, cs],
                                            in1=dy[:, cs], op=OP.mult)
                    nc.vector.tensor_tensor(out=d2[:, cs], in0=d2[:, cs],
                                            in1=dy[:, cs], op=OP.add)
                    nc.scalar.activation(dist[:, cs], d2[:, cs], AF.Sqrt,
                                         bias=zb[:, 0:1])
                    nc.scalar.activation(ex[:, cs], dist[:, cs], AF.Exp,
                                         bias=zb[:, 0:1], scale=-1.0)

                half(0)
                half(1)

                # labels for the final step only (cols 224:256)
                FS = (D - 1) * K
                l01 = wpool.tile([128, K], f32, tag="l01")
                l23 = wpool.tile([128, K], f32, tag="l23")
                lbl = wpool.tile([128, K], f32, tag="lbl")
                nc.vector.select(l01[:], m0[:, FS:, 0], gt[:, FS:, 6],
                                 gt[:, FS:, 2])
                nc.vector.select(l23[:], m0[:, FS:, 0], gt[:, FS:, 14],
                                 gt[:, FS:, 10])
                nc.vector.select(lbl[:], m1[:, FS:, 0], l23[:], l01[:])

                # per-(b,d) softmax stats
                S = wpool.tile([128, 8], f32, tag="S")
                M = wpool.tile([128, 8], f32, tag="M")
                exv = ex[:].rearrange("p (d k) -> p d k", k=K)
                nc.vector.tensor_reduce(out=S[:], in_=exv, axis=AX.X, op=OP.add)
                nc.vector.tensor_reduce(out=M[:], in_=exv, axis=AX.X, op=OP.max)
                rS = wpool.tile([128, 8], f32, tag="rS")
                nc.vector.reciprocal(rS[:], S[:])
                rat = wpool.tile([128, 8], f32, tag="rat")
                nc.vector.tensor_tensor(out=rat[:], in0=M[:], in1=rS[:],
                                        op=OP.mult)
                lg = wpool.tile([128, 8], f32, tag="lg")
                nc.scalar.activation(lg[:], rat[:], AF.Ln, bias=eb[:, 0:1])
                t1 = wpool.tile([128, 1], f32, tag="t1")
                nc.vector.tensor_reduce(out=t1[:], in_=lg[:, 0:7], axis=AX.X,
                                        op=OP.add)

                # final step: p[b, c] = sum_k ex[b, 7, k] * (lbl[b, k] == c)
                # via [128, C, K] broadcast compare/mult + reduce over K
                p = wpool.tile([128, C], f32, tag="p")
                eqm = wpool.tile([128, C, K], f32, tag="eqm")
                iot_b = iot[:].rearrange("p (c o) -> p c o", o=1).to_broadcast(
                    [128, C, K])
                lbl_b = lbl[:].rearrange("p (o k) -> p o k", o=1).to_broadcast(
                    [128, C, K])
                ex7_b = ex[:, FS:].rearrange("p (o k) -> p o k",
                                             o=1).to_broadcast([128, C, K])
                nc.vector.tensor_tensor(out=eqm[:], in0=iot_b, in1=lbl_b,
                                        op=OP.is_equal)
                nc.vector.tensor_tensor(out=eqm[:], in0=eqm[:], in1=ex7_b,
                                        op=OP.mult)
                nc.vector.tensor_reduce(out=p[:], in_=eqm[:], axis=AX.X,
                                        op=OP.add)

                # out = log(p / S7 + eps) + term1
                nc.vector.tensor_scalar(out=p[:], in0=p[:], scalar1=rS[:, 7:8],
                                        scalar2=None, op0=OP.mult)
                ot = wpool.tile([128, C], f32, tag="ot")
                nc.scalar.activation(ot[:], p[:], AF.Ln, bias=eb[:, 0:1])
                nc.vector.tensor_scalar(out=ot[:], in0=ot[:], scalar1=t1[:, 0:1],
                                        scalar2=None, op0=OP.add)
                nc.sync.dma_start(out=y[128 * t:128 * (t + 1), :], in_=ot[:])

    nc.compile()
    return nc


def prep_inputs(x, node_data, W1, b1, W2, b2, W3, b3, W4, b4, labels, nbr_idx,
                npad=NPAD):
    """Host-side sharding + integer index/layout prep. Returns per-core maps."""
    shard = npad // 8
    sapt = shard // 128
    ndTf = np.zeros((2, npad), np.float32)
    ndTf[:, :N] = node_data.T
    lblf = np.zeros((2, npad), np.float32)
    lblf[0, :N] = labels.astype(np.float32)
    iota = np.broadcast_to(np.arange(C, dtype=np.float32), (128, C)).copy()

    # gather row/sub for every (b, d, k): pos = c*shard + (l%128)*sapt + l//128
    v = nbr_idx.astype(np.int64)                      # [B, D, K]
    own = v // shard
    l = v - own * shard
    pos = own * shard + (l % 128) * sapt + (l // 128)
    row = (pos >> 2).astype(np.int16)
    sub = (pos & 3).astype(np.uint8)

    common = {
        "w1t": np.ascontiguousarray(W1.T), "w2t": np.ascontiguousarray(W2.T),
        "w3t": np.ascontiguousarray(W3.T), "w4t": np.ascontiguousarray(W4.T),
        "b1": b1.reshape(-1, 1).astype(np.float32),
        "b2": b2.reshape(-1, 1).astype(np.float32),
        "b3": b3.reshape(-1, 1).astype(np.float32),
        "b4": b4.reshape(-1, 1).astype(np.float32),
        "iota": iota,
    }

    maps = []
    ntiles = NTILES
    for c in range(NCORES):
        bsl = slice(BC * c, BC * (c + 1))
        # k-order within a tile: k = dk*128 + b_loc ; call q covers k in
        # [Pq, Pq+P), wrapped [16, P//16] per call (P = PER_CALL).
        rows_c = row[bsl].reshape(ntiles, 128, D * K)      # [t, b_loc, dk]
        subs_c = sub[bsl].reshape(ntiles, 128, D * K)
        kord = np.transpose(rows_c, (0, 2, 1)).reshape(ntiles, D * K * 128)
        # k = dk*128 + b_loc -> kord[t, k]
        wid = np.zeros((ntiles, 16, 2048), np.int16)
        kk = np.arange(D * K * 128)
        q = kk // PER_CALL
        kp = kk % PER_CALL
        wid[:, kp % 16, (PER_CALL // 16) * q + kp // 16] = kord[:, kk]
        # masks 2-wide: [t, b_loc, dk, 0:2] = bit
        m0 = np.zeros((ntiles, 128, D * K, 2), np.int8)
        m1 = np.zeros((ntiles, 128, D * K, 2), np.int8)
        m0[...] = (subs_c & 1).astype(np.int8)[..., None]
        m1[...] = (subs_c >> 1).astype(np.int8)[..., None]
        m0 = m0.reshape(ntiles, 128, 512)
        m1 = m1.reshape(ntiles, 128, 512)
        maps.append(dict(common,
                         ndT=np.ascontiguousarray(ndTf[:, shard * c:shard * (c + 1)]),
                         lblfT=np.ascontiguousarray(lblf[:, shard * c:shard * (c + 1)]),
                         xT=np.ascontiguousarray(x[bsl].T),
                         widx=wid, m0e=m0, m1e=m1))
    return maps


_NC_CACHE = {}


def kernel(**inputs):
    _install_ntff_shim()
    from concourse.bass_utils import run_bass_kernel_spmd

    if "nc" not in _NC_CACHE:
        _NC_CACHE["nc"] = build_kernel()
    nc = _NC_CACHE["nc"]

    maps = prep_inputs(**inputs)
    res = run_bass_kernel_spmd(nc, maps, list(range(NCORES)), trace=False)
    out = np.concatenate([res.results[c]["y"] for c in range(NCORES)], axis=0)
    return out


if __name__ == "__main__":
    import reference
    inputs = {k: np.asarray(v) for k, v in reference.setup_inputs().items()}
    got = kernel(**inputs)
    exp = np.asarray(reference.reference(**inputs))
    rel = np.abs(got - exp) / np.maximum(np.abs(exp), 1e-6)
    print("max rel:", rel.max(), "mean rel:", rel.mean())



# revision 33
# speedup vs baseline: 1.0574x; 1.0574x over previous
"""DeepBoundaryTree retrieval-knn kernel for 8x Trainium2 (Bass/Tile).

Strategy: data-parallel over queries B across 8 cores. Each core:
  - runs the 2-100-100-30-2 MLP over its 1/8 shard of (padded) nodes on the
    tensor engine, AllGathers the 16B records [x', y', label, 0] and spreads
    them into a DRAM table with 4 records per 256B-strided row
    (int16-addressable by dma_gather),
  - gathers the 256 neighbor records per query with dma_gather, round-robined
    across 4 SWDGE queues (num_swdge_queues=4) so descriptor generation runs
    on all four Q7 core pairs concurrently (~4x gather throughput; desc-gen
    at ~8.4 ns/index is the kernel bottleneck). 1024 idx/call is the SWDGE
    ring limit (128 descs/engine/queue) - larger calls deadlock,
  - selects the right 16B record 1-of-4 via precomputed bit masks, computes
    pairwise distance + per-(b,d) softmax stats on vector/scalar engines,
  - computes the final-step class sums with 3 broadcast DVE ops on
    [128, C, K] (iota==label * exp weights, reduce over K),
  - writes its [1024, 100] output slice.

Host side only reshapes/shards inputs and precomputes integer index/layout
tensors derived from nbr_idx (gather rows, sub-slot select masks).
"""
import sys
import types
import numpy as np

B, N, D, K, C = 8192, 100000, 8, 32, 100
NCORES = 8
BC = B // NCORES            # queries per core
NTILES = BC // 128          # 128-query tiles per core
NPAD = 102400               # nodes padded to 8*128*100 (sapt%4==0 so any
                            # 4-slot boundary is whole-table-row aligned)
ROWS = NPAD // 4            # 25600 table rows (256B stride, 4 records each)
ROW_F32 = 64                # 64 f32 = 256B row stride
PER_CALL = 1024             # dma_gather indices per call (hard SWDGE ring
                            # limit: 128 descs/engine/queue, runtime-fixed)
CALLS_PER_TILE = (128 * D * K) // PER_CALL  # 32
APT = NPAD // 128           # 784 free-dim slots per partition in packed recs
EPS_PD = 1e-6
EPS_LOG = 1e-4


def _install_ntff_shim():
    try:
        import antenv
        if "antenv.axon_hooks" in sys.modules:
            return
        mod = types.ModuleType("antenv.axon_hooks")
        holder = [None]
        mod.set_axon_ntff_profile_hook = lambda h: holder.__setitem__(0, h)
        mod.get_axon_ntff_profile_hook = lambda: holder[0]
        sys.modules["antenv.axon_hooks"] = mod
        antenv.axon_hooks = mod
        import trn_agent_boot.trn_boot as tb
        mod.set_axon_ntff_profile_hook(
            tb._ntff_profile_via_ctypes("/opt/axon/libaxon_pjrt.so"))
    except Exception:
        pass


def dma_gather_raw(eng, mybir, out_ap, in_ap, idxs_ap, num_idxs, elem_size,
                   elem_step, queue_num=0):
    """bass.dma_gather without the 256B-element assert (DRAM src, no transpose)."""
    stride_bytes = elem_step * mybir.dt.size(in_ap.dtype)
    stride_bytes_256 = stride_bytes // 256
    assert stride_bytes_256 * 256 == stride_bytes and stride_bytes_256 < 256
    _in_ap = eng.lower_ap_dma(in_ap, for_custom_bir_dma=True)
    _idxs_ap = eng.lower_ap(idxs_ap)
    _out_ap = eng.lower_ap(out_ap)
    return eng.add_instruction(
        mybir.InstDMAGatherAnt(
            name=eng.bass.get_next_instruction_name(),
            ins=[*_in_ap, _idxs_ap, eng.lower_val_access(eng.to_reg(num_idxs))],
            outs=[_out_ap],
            transpose=False,
            num_idxs=num_idxs,
            elem_size=elem_size,
            stride_bytes_256=stride_bytes_256,
            gen_mode=0,
            single_packet=True,
            queue_num=queue_num,
            sbuf_tokens_per_rank=0,
            sbuf_free_dim_per_rank=0,
            sbuf_free_dim_pad_per_rank=0,
            sbuf_byte_offset=0,
        ))


def build_kernel(npad=NPAD, ntiles=NTILES):
    import concourse.bacc as bacc
    import concourse.mybir as mybir
    from concourse.tile import TileContext
    from concourse.masks import make_identity

    apt = npad // 128
    rows = npad // 4
    f32 = mybir.dt.float32
    AX = mybir.AxisListType
    AF = mybir.ActivationFunctionType
    OP = mybir.AluOpType

    nc = bacc.Bacc("TRN2", target_bir_lowering=False, debug=False,
                   num_swdge_queues=4)

    # ---- I/O ----
    shard = npad // 8
    sapt = shard // 128
    ndT = nc.dram_tensor("ndT", [2, shard], f32, kind="ExternalInput")
    lblfT = nc.dram_tensor("lblfT", [2, shard], f32, kind="ExternalInput")
    SA = 64                     # a-slots in the early AllGather: largest A
                                # whose collective still ends by MLP-end
                                # (measured: SA=72 ended 8us past MLP-end)
    SB = sapt - SA              # 32 a-slots in the late small AllGather
    packed_cA = nc.dram_tensor("packed_cA", [128, SA * 4], f32)
    packed_cB = nc.dram_tensor("packed_cB", [128, SB * 4], f32)
    packed_allA = nc.dram_tensor("packed_allA", [8, 128, SA * 4], f32,
                                 addr_space="Shared")
    packed_allB = nc.dram_tensor("packed_allB", [8, 128, SB * 4], f32,
                                 addr_space="Shared")
    xT = nc.dram_tensor("xT", [2, 128 * ntiles], f32, kind="ExternalInput")
    w1t = nc.dram_tensor("w1t", [2, 100], f32, kind="ExternalInput")
    w2t = nc.dram_tensor("w2t", [100, 100], f32, kind="ExternalInput")
    w3t = nc.dram_tensor("w3t", [100, 30], f32, kind="ExternalInput")
    w4t = nc.dram_tensor("w4t", [30, 2], f32, kind="ExternalInput")
    b1 = nc.dram_tensor("b1", [100, 1], f32, kind="ExternalInput")
    b2 = nc.dram_tensor("b2", [100, 1], f32, kind="ExternalInput")
    b3 = nc.dram_tensor("b3", [30, 1], f32, kind="ExternalInput")
    b4 = nc.dram_tensor("b4", [2, 1], f32, kind="ExternalInput")
    iota = nc.dram_tensor("iota", [128, C], f32, kind="ExternalInput")
    widx = nc.dram_tensor("widx", [ntiles, 16, 2048], mybir.dt.int16,
                          kind="ExternalInput")
    m0e = nc.dram_tensor("m0e", [ntiles, 128, 512], mybir.dt.int8, kind="ExternalInput")
    m1e = nc.dram_tensor("m1e", [ntiles, 128, 512], mybir.dt.int8, kind="ExternalInput")
    y = nc.dram_tensor("y", [128 * ntiles, C], f32, kind="ExternalOutput")

    table = nc.dram_tensor("table", [rows, ROW_F32], f32)

    with TileContext(nc) as tc:
        with tc.tile_pool(name="const", bufs=1) as cpool, \
             tc.tile_pool(name="mlp", bufs=4) as mpool, \
             tc.tile_pool(name="recs", bufs=1) as rpool, \
             tc.tile_pool(name="gath", bufs=2) as gpool, \
             tc.tile_pool(name="work", bufs=2) as wpool, \
             tc.tile_pool(name="psA", bufs=2, space="PSUM") as pA, \
             tc.tile_pool(name="psB", bufs=1, space="PSUM") as pB, \
             tc.tile_pool(name="pt", bufs=2, space="PSUM") as tpool:

            # ---- constants ----
            w1s = cpool.tile([2, 100], f32)
            w2s = cpool.tile([100, 100], f32)
            w3s = cpool.tile([100, 30], f32)
            w4s = cpool.tile([30, 2], f32)
            b1s = cpool.tile([100, 1], f32)
            b2s = cpool.tile([100, 1], f32)
            b3s = cpool.tile([30, 1], f32)
            b4s = cpool.tile([2, 1], f32)
            iot = cpool.tile([128, C], f32)
            id4 = cpool.tile([4, 4], f32)
            zb = cpool.tile([128, 1], f32)
            eb = cpool.tile([128, 1], f32)
            nc.vector.memset(zb[:], 0.0)
            nc.vector.memset(eb[:], float(EPS_LOG))
            for dst, src in ((w1s, w1t), (w2s, w2t), (w3s, w3t), (w4s, w4t),
                             (b1s, b1), (b2s, b2), (b3s, b3), (b4s, b4),
                             (iot, iota)):
                nc.sync.dma_start(out=dst[:], in_=src[:])
            make_identity(nc, id4[:])
            f32r = mybir.dt.float32r
            w1r = cpool.tile([2, 100], f32r)
            w2r = cpool.tile([100, 100], f32r)
            w3r = cpool.tile([100, 30], f32r)
            w4r = cpool.tile([30, 2], f32r)
            for dstr, srcr in ((w1r, w1s), (w2r, w2s), (w3r, w3s), (w4r, w4s)):
                nc.vector.tensor_copy(out=dstr[:], in_=srcr[:])

            xTs = cpool.tile([2, 128 * ntiles], f32)
            nc.sync.dma_start(out=xTs[:], in_=xT[:])

            recs = rpool.tile([128, sapt, 4], f32)
            qxT = rpool.tile([2, 128 * ntiles], f32)

            # ---- MLP over nodes (and queries appended) in 512-col steps ----
            def mlp_steps(src_dram, src_tile, ncols, out_cb, base=0):
                nsteps = (ncols + 511) // 512
                for s in range(nsteps):
                    w = min(512, ncols - 512 * s)
                    cs = slice(base + 512 * s, base + 512 * s + w)
                    if src_dram is not None:
                        nds = mpool.tile([2, 512], f32, tag="nds")
                        nc.sync.dma_start(out=nds[:, :w], in_=src_dram[:, cs])
                        srcv = nds[:, :w]
                    else:
                        srcv = src_tile[:, cs]
                    srcr = mpool.tile([2, 512], f32r, tag="srcr")
                    nc.vector.tensor_copy(out=srcr[:, :w], in_=srcv)
                    ps1 = pA.tile([100, 512], f32, tag="ps1")
                    nc.tensor.matmul(ps1[:, :w], w1r[:], srcr[:, :w],
                                     start=True, stop=True)
                    h1 = mpool.tile([100, 512], f32r, tag="h1")
                    nc.scalar.activation(h1[:, :w], ps1[:, :w], AF.Relu,
                                         bias=b1s[:, 0:1])
                    ps2 = pA.tile([100, 512], f32, tag="ps2")
                    nc.tensor.matmul(ps2[:, :w], w2r[:], h1[:, :w],
                                     start=True, stop=True)
                    h2 = mpool.tile([100, 512], f32r, tag="h2")
                    # relu on DVE: (ps2 + b2) max 0
                    nc.vector.tensor_scalar(
                        out=h2[:, :w], in0=ps2[:, :w], scalar1=b2s[:, 0:1],
                        scalar2=0.0, op0=OP.add, op1=OP.max)
                    ps3 = pB.tile([30, 512], f32, tag="ps3")
                    nc.tensor.matmul(ps3[:, :w], w3r[:], h2[:, :w],
                                     start=True, stop=True)
                    h3 = mpool.tile([30, 512], f32r, tag="h3")
                    nc.scalar.activation(h3[:, :w], ps3[:, :w], AF.Relu,
                                         bias=b3s[:, 0:1])
                    ps4 = pB.tile([2, 512], f32, tag="ps4")
                    nc.tensor.matmul(ps4[:, :w], w4r[:], h3[:, :w],
                                     start=True, stop=True)
                    out_cb(s, ps4, w)

            # nodes: build records, transposes lag one step so PE never
            # stalls on the DVE record build of the same step
            pending = []

            def flush_rec(rec512, s, w):
                pst = tpool.tile([128, 16], f32, tag="pst")
                nb = w // 128
                for j in range(nb):
                    nc.tensor.transpose(pst[:, 4 * j:4 * (j + 1)],
                                        rec512[:, 128 * j:128 * (j + 1)],
                                        id4[:])
                nc.vector.tensor_copy(
                    out=recs[:, 4 * s:4 * s + nb, :].rearrange(
                        "p a c -> p (a c)"),
                    in_=pst[:, :4 * nb])

            def node_out(s, ps4, w):
                rec512 = mpool.tile([4, 512], f32, tag="rec512")
                # x' = x + b4 + eps  (vector: (ps4 + b4) + eps)
                nc.vector.tensor_scalar(
                    out=rec512[0:2, :w], in0=ps4[:, :w], scalar1=b4s[:, 0:1],
                    scalar2=float(EPS_PD), op0=OP.add, op1=OP.add)
                nc.sync.dma_start(out=rec512[2:4, :w],
                                  in_=lblfT[:, 512 * s:512 * s + w])
                pending.append((rec512, s, w))
                if len(pending) >= 2:
                    flush_rec(*pending.pop(0))

            # part A (17 steps = 68 slots): AllGather it while the last 8
            # MLP steps still run, so its ~20us fixed cost is hidden
            stA = SA // 4
            mlp_steps(ndT, None, stA * 512, node_out)
            while pending:
                flush_rec(*pending.pop(0))
            nc.sync.dma_start(out=packed_cA[:],
                              in_=recs[:, 0:SA, :].rearrange(
                                  "p a c -> p (a c)"))
            nc.gpsimd.collective_compute(
                "AllGather", OP.bypass,
                replica_groups=[list(range(8))],
                ins=[packed_cA[:]],
                outs=[packed_allA[:]],
            )
            tb_v = table[:, 0:16].rearrange("(c p v) e -> c p v e", c=8,
                                            p=128)
            pkA = packed_allA[:].rearrange("c p (r e) -> c p r e", e=16)
            rA = SA // 8                # rows per half-chunk spread DMA
            for i in range(8):
                for h in range(2):
                    nc.gpsimd.dma_start(
                        out=tb_v[i, :, rA * h:rA * (h + 1), :],
                        in_=pkA[i, :, rA * h:rA * (h + 1), :])

            mlp_steps(ndT, None, shard - stA * 512,
                      lambda s, ps4, w: node_out(s + stA, ps4, w),
                      base=stA * 512)
            while pending:
                flush_rec(*pending.pop(0))
            nc.sync.dma_start(out=packed_cB[:],
                              in_=recs[:, SA:, :].rearrange(
                                  "p a c -> p (a c)"))
            nc.gpsimd.collective_compute(
                "AllGather", OP.bypass,
                replica_groups=[list(range(8))],
                ins=[packed_cB[:]],
                outs=[packed_allB[:]],
            )
            pkB = packed_allB[:].rearrange("c p (r e) -> c p r e", e=16)
            for i in range(8):
                nc.sync.dma_start(out=tb_v[i, :, SA // 4:sapt // 4, :],
                                  in_=pkB[i, :, :, :])

            # queries: qxT = out + b4 (no eps)
            def query_out(s, ps4, w):
                nc.vector.tensor_scalar(
                    out=qxT[:, 512 * s:512 * s + w], in0=ps4[:, :w],
                    scalar1=b4s[:, 0:1], scalar2=None, op0=OP.add)

            mlp_steps(None, xTs, 128 * ntiles, query_out)

            # per-tile query coords [128, 2]
            id2 = cpool.tile([2, 2], f32)
            make_identity(nc, id2[:])

            # ---- main loop over 128-query tiles ----
            for t in range(ntiles):
                qps = tpool.tile([128, 16], f32, tag="pst")
                nc.tensor.transpose(qps[:, 0:2], qxT[:, 128 * t:128 * (t + 1)],
                                    id2[:])
                qx = wpool.tile([128, 2], f32, tag="qx")
                nc.vector.tensor_copy(out=qx[:], in_=qps[:, 0:2])

                idx_t = gpool.tile([128, 2048], mybir.dt.int16, tag="idx")
                for g in range(8):
                    nc.sync.dma_start(out=idx_t[16 * g:16 * (g + 1), :],
                                      in_=widx[t, :, :])
                m0 = wpool.tile([128, 256, 2], mybir.dt.int8, tag="m0")
                m1 = wpool.tile([128, 256, 2], mybir.dt.int8, tag="m1")
                nc.sync.dma_start(out=m0[:].rearrange("p s c -> p (s c)"),
                                  in_=m0e[t, :, :])
                nc.sync.dma_start(out=m1[:].rearrange("p s c -> p (s c)"),
                                  in_=m1e[t, :, :])

                gt = gpool.tile([128, 256, 16], f32, tag="gt")
                GC = PER_CALL // 128   # gt cols per call
                IC = PER_CALL // 16    # idx cols per call
                s01t = wpool.tile([128, 256, 2], f32, tag="s01")
                s23t = wpool.tile([128, 256, 2], f32, tag="s23")
                rect = wpool.tile([128, 256, 2], f32, tag="rec")
                dx = wpool.tile([128, 256], f32, tag="dx")
                dy = wpool.tile([128, 256], f32, tag="dy")
                d2 = wpool.tile([128, 256], f32, tag="d2")
                dist = wpool.tile([128, 256], f32, tag="dist")
                ex = wpool.tile([128, 256], f32, tag="ex")
                HB = CALLS_PER_TILE // 2

                def half(h):
                    # gathers for this half, then its select/dist/exp so only
                    # the second half's chain is exposed after the last call
                    for q in range(HB * h, HB * (h + 1)):
                        dma_gather_raw(
                            nc.gpsimd, mybir,
                            gt[:, GC * q:GC * (q + 1), :],
                            table[:, 0:16],
                            idx_t[:, IC * q:IC * (q + 1)],
                            num_idxs=PER_CALL, elem_size=16,
                            elem_step=ROW_F32, queue_num=q % 4)
                    cs = slice(128 * h, 128 * (h + 1))
                    nc.vector.select(s01t[:, cs, :], m0[:, cs, :],
                                     gt[:, cs, 4:6], gt[:, cs, 0:2])
                    nc.vector.select(s23t[:, cs, :], m0[:, cs, :],
                                     gt[:, cs, 12:14], gt[:, cs, 8:10])
                    nc.vector.select(rect[:, cs, :], m1[:, cs, :],
                                     s23t[:, cs, :], s01t[:, cs, :])
                    nc.vector.tensor_scalar(out=dx[:, cs], in0=rect[:, cs, 0],
                                            scalar1=qx[:, 0:1], scalar2=None,
                                            op0=OP.subtract)
                    nc.vector.tensor_scalar(out=dy[:, cs], in0=rect[:, cs, 1],
                                            scalar1=qx[:, 1:2], scalar2=None,
                                            op0=OP.subtract)
                    nc.vector.tensor_tensor(out=d2[:, cs], in0=dx[:, cs],
                                            in1=dx[:, cs], op=OP.mult)
                    nc.vector.tensor_tensor(out=dy[:, cs], in0=dy[:, cs],
                                            in1=dy[:, cs], op=OP.mult)
                    nc.vector.tensor_tensor(out=d2[:, cs], in0=d2[:, cs],
                                            in1=dy[:, cs], op=OP.add)
                    nc.scalar.activation(dist[:, cs], d2[:, cs], AF.Sqrt,
                                         bias=zb[:, 0:1])
                    nc.scalar.activation(ex[:, cs], dist[:, cs], AF.Exp,
                                         bias=zb[:, 0:1], scale=-1.0)

                half(0)
                half(1)

                # labels for the final step only (cols 224:256)
                FS = (D - 1) * K
                l01 = wpool.tile([128, K], f32, tag="l01")
                l23 = wpool.tile([128, K], f32, tag="l23")
                lbl = wpool.tile([128, K], f32, tag="lbl")
                nc.vector.select(l01[:], m0[:, FS:, 0], gt[:, FS:, 6],
                                 gt[:, FS:, 2])
                nc.vector.select(l23[:], m0[:, FS:, 0], gt[:, FS:, 14],
                                 gt[:, FS:, 10])
                nc.vector.select(lbl[:], m1[:, FS:, 0], l23[:], l01[:])

                # per-(b,d) softmax stats
                S = wpool.tile([128, 8], f32, tag="S")
                M = wpool.tile([128, 8], f32, tag="M")
                exv = ex[:].rearrange("p (d k) -> p d k", k=K)
                nc.vector.tensor_reduce(out=S[:], in_=exv, axis=AX.X, op=OP.add)
                nc.vector.tensor_reduce(out=M[:], in_=exv, axis=AX.X, op=OP.max)
                rS = wpool.tile([128, 8], f32, tag="rS")
                nc.vector.reciprocal(rS[:], S[:])
                rat = wpool.tile([128, 8], f32, tag="rat")
                nc.vector.tensor_tensor(out=rat[:], in0=M[:], in1=rS[:],
                                        op=OP.mult)
                lg = wpool.tile([128, 8], f32, tag="lg")
                nc.scalar.activation(lg[:], rat[:], AF.Ln, bias=eb[:, 0:1])
                t1 = wpool.tile([128, 1], f32, tag="t1")
                nc.vector.tensor_reduce(out=t1[:], in_=lg[:, 0:7], axis=AX.X,
                                        op=OP.add)

                # final step: p[b, c] = sum_k ex[b, 7, k] * (lbl[b, k] == c)
                # via [128, C, K] broadcast compare/mult + reduce over K
                p = wpool.tile([128, C], f32, tag="p")
                eqm = wpool.tile([128, C, K], f32, tag="eqm")
                iot_b = iot[:].rearrange("p (c o) -> p c o", o=1).to_broadcast(
                    [128, C, K])
                lbl_b = lbl[:].rearrange("p (o k) -> p o k", o=1).to_broadcast(
                    [128, C, K])
                ex7_b = ex[:, FS:].rearrange("p (o k) -> p o k",
                                             o=1).to_broadcast([128, C, K])
                nc.vector.tensor_tensor(out=eqm[:], in0=iot_b, in1=lbl_b,
                                        op=OP.is_equal)
                nc.vector.tensor_tensor(out=eqm[:], in0=eqm[:], in1=ex7_b,
                                        op=OP.mult)
                nc.vector.tensor_reduce(out=p[:], in_=eqm[:], axis=AX.X,
                                        op=OP.add)

                # out = log(p / S7 + eps) + term1
                nc.vector.tensor_scalar(out=p[:], in0=p[:], scalar1=rS[:, 7:8],
                                        scalar2=None, op0=OP.mult)
                ot = wpool.tile([128, C], f32, tag="ot")
                nc.scalar.activation(ot[:], p[:], AF.Ln, bias=eb[:, 0:1])
                nc.vector.tensor_scalar(out=ot[:], in0=ot[:], scalar1=t1[:, 0:1],
                                        scalar2=None, op0=OP.add)
                nc.sync.dma_start(out=y[128 * t:128 * (t + 1), :], in_=ot[:])

    nc.compile()
    return nc


def prep_inputs(x, node_data, W1, b1, W2, b2, W3, b3, W4, b4, labels, nbr_idx,
                npad=NPAD):
    """Host-side sharding + integer index/layout prep. Returns per-core maps."""
    shard = npad // 8
    sapt = shard // 128
    ndTf = np.zeros((2, npad), np.float32)
    ndTf[:, :N] = node_data.T
    lblf = np.zeros((2, npad), np.float32)
    lblf[0, :N] = labels.astype(np.float32)
    iota = np.broadcast_to(np.arange(C, dtype=np.float32), (128, C)).copy()

    # gather row/sub for every (b, d, k): pos = c*shard + (l%128)*sapt + l//128
    v = nbr_idx.astype(np.int64)                      # [B, D, K]
    own = v // shard
    l = v - own * shard
    pos = own * shard + (l % 128) * sapt + (l // 128)
    row = (pos >> 2).astype(np.int16)
    sub = (pos & 3).astype(np.uint8)

    common = {
        "w1t": np.ascontiguousarray(W1.T), "w2t": np.ascontiguousarray(W2.T),
        "w3t": np.ascontiguousarray(W3.T), "w4t": np.ascontiguousarray(W4.T),
        "b1": b1.reshape(-1, 1).astype(np.float32),
        "b2": b2.reshape(-1, 1).astype(np.float32),
        "b3": b3.reshape(-1, 1).astype(np.float32),
        "b4": b4.reshape(-1, 1).astype(np.float32),
        "iota": iota,
    }

    maps = []
    ntiles = NTILES
    for c in range(NCORES):
        bsl = slice(BC * c, BC * (c + 1))
        # k-order within a tile: k = dk*128 + b_loc ; call q covers k in
        # [Pq, Pq+P), wrapped [16, P//16] per call (P = PER_CALL).
        rows_c = row[bsl].reshape(ntiles, 128, D * K)      # [t, b_loc, dk]
        subs_c = sub[bsl].reshape(ntiles, 128, D * K)
        kord = np.transpose(rows_c, (0, 2, 1)).reshape(ntiles, D * K * 128)
        # k = dk*128 + b_loc -> kord[t, k]
        wid = np.zeros((ntiles, 16, 2048), np.int16)
        kk = np.arange(D * K * 128)
        q = kk // PER_CALL
        kp = kk % PER_CALL
        wid[:, kp % 16, (PER_CALL // 16) * q + kp // 16] = kord[:, kk]
        # masks 2-wide: [t, b_loc, dk, 0:2] = bit
        m0 = np.zeros((ntiles, 128, D * K, 2), np.int8)
        m1 = np.zeros((ntiles, 128, D * K, 2), np.int8)
        m0[...] = (subs_c & 1).astype(np.int8)[..., None]
        m1[...] = (subs_c >> 1).astype(np.int8)[..., None]
        m0 = m0.reshape(ntiles, 128, 512)
        m1 = m1.reshape(ntiles, 128, 512)
        maps.append(dict(common,
                         ndT=np.ascontiguousarray(ndTf[:, shard * c:shard * (c + 1)]),
                         lblfT=np.ascontiguousarray(lblf[:, shard * c:shard * (c + 1)]),
                         xT=np.ascontiguousarray(x[bsl].T),
                         widx=wid, m0e=m0, m1e=m1))
    return maps


_NC_CACHE = {}


def kernel(**inputs):
    _install_ntff_shim()
    from concourse.bass_utils import run_bass_kernel_spmd

    if "nc" not in _NC_CACHE:
        _NC_CACHE["nc"] = build_kernel()
    nc = _NC_CACHE["nc"]

    maps = prep_inputs(**inputs)
    res = run_bass_kernel_spmd(nc, maps, list(range(NCORES)), trace=False)
    out = np.concatenate([res.results[c]["y"] for c in range(NCORES)], axis=0)
    return out


if __name__ == "__main__":
    import reference
    inputs = {k: np.asarray(v) for k, v in reference.setup_inputs().items()}
    got = kernel(**inputs)
    exp = np.asarray(reference.reference(**inputs))
    rel = np.abs(got - exp) / np.maximum(np.abs(exp), 1e-6)
    print("max rel:", rel.max(), "mean rel:", rel.mean())

